# revision 24
# baseline (speedup 1.0000x reference)
"""BatchHardTripletLoss on 8 Trainium2 NeuronCores (Bass/Tile), v2.2.

Sharding: embeddings row-sharded 8 ways; each core computes its
[1024, 8192] slab of psum[i,j] = -2*x_i.x_j with fp16 matmuls and
reduces it on-device; per-row (negmin, posmax) go back to the host,
which finishes sq_i + sqrt + relu + mean (O(N) work).

Layout: rows are pre-sorted by label on host. Each core's column
stream is rotated PER TILE (host pads the rotated arrays by 1024 cols
so every device slice is contiguous): tile t reads columns starting at
global col cR + 128t + 192, which lands the tile's same-label window
in the LAST 512 columns of its 8192-col sweep. Everything before that
is guaranteed different-label, so the hardest-negative reduction runs
as a fused custom DVE op ADD_MIN (cand = psum + sq_j broadcast, with a
chained min accumulator). The eq zone (final 512 cols, sq_j via a K=1
accumulate matmul) uses the eq-masked custom ops CAND_MIN/CAND_MAX;
the self-pair anchors no-positive detection via TAU.

v2.2 structure (106.9us -> 101.8us measured):
- 4 chunks of 2048 per tile (psum = 2 bufs x [128,2048] = all 8
  banks); the eq zone rides in the tail 512 of chunk 3, dropping the
  separate eq psum pool, the xt_eq slice, and one DVE op per tile.
- CHUNK-INDEX-MAJOR sweep (all 8 tiles' chunk c, then c+1): each
  landed input slice (x_c on sync queue, sqb_c broadcast on scalar
  queue, ~750KB each at ~110-125KB/us/queue) unlocks ~17us of DVE
  work, so after chunk 0 the two DMA queues stay ahead and the DVE
  runs gap-free at its 1 elem/cycle/lane roofline. Tile-major order
  (v2) instead stalled ~2us per early chunk and started 6us later.
  Per-tile accumulators live across sweeps (chains dict, acc pool).
- Tile 0's chunk 0 is split 512+512+1024 with matching split DMA
  pieces (subtile deps are tracked, so no duplicate fast-start tiles
  are needed), and the weight slab loads as [0:384] early on sync
  (tiles 0-2) + [384:1024] late on scalar: the first reduction starts
  ~13us and tile 1 unblocks ~2us earlier (98.5-99.9us measured).
- The eq tail is staged psum->SBUF fp16 by the otherwise-idle scalar
  engine so the big psum buf frees right after the plain 1536-wide
  reduction; without this the CAND ops hold the buf and the PE
  pipeline bubbles ~3.4us per 2 tiles in the chunk-3 sweep (measured).
- Remaining non-compute time: ~8us preamble+DMA-spinup to first op,
  ~4us of supply-rate-bound lead-in gaps, ~3us postamble ceremony.
  DVE busy ~81us (66us streaming + ~115ns/op overhead x 49 ops +
  CAND/merge extras) is the roofline wall; see the 2x notes below for
  why a second reduce lane is blocked.

Notes from HW profiling (all verified on device this series):
- tensor_tensor_reduce (stock DVE) with a PSUM input crashes the
  device even with one PSUM operand; the compiler verifier separately
  rejects two PSUM reads (NCC_IBVF027). Custom DVE ops with in0=psum,
  in1=SBUF are the proven pattern, at 1 elem/cycle/lane (0.96 GHz).
- Matmuls stream at ~295ns/512 cols when the PE is saturated but
  ~600ns in this DVE-bound kernel (HAM throttle never lifts), which
  is why scalar-engine-staged pair-min (needs K=1 sq matmuls) lost in
  three measured attempts.
- gpsimd tensor_reduce supports only partition-axis (C/XYZWC) - no
  free-axis reduction offload; gpsimd-issued DMAs use slow software
  DGE. The activation engine copies psum->SBUF fine but has no min.
- Startup is ~6us fixed preamble + ~2.5us DMA-queue spin-up before
  any transfer; whole-tile DMA dependencies (not bandwidth) gated the
  first reduction until inputs were split per chunk index. v2.1: the
  first ADD_MIN additionally waits on queue predecessors (sqrow +
  m2slab + full xts[0] ~ 896KB on one queue => first op at ~18.9us).
  A fast-start restructure (narrow 512-col tile-0 chunk, dedicated
  m2a/xfirst/sqbfirst tiles loaded first) DID pull the first op to
  12.6us, but serializing all x-slices on sync / sqb-slices on scalar
  starved later chunks (7.3us stall at tile0-chunk3): net 111.6us vs
  107.1us, so it was reverted. Only SP and Activation can initiate
  HWDGE DMA (vector cannot). The early phase is DMA-rate-bound
  (~110-125KB/us per queue, ~6.7MB of inputs); a real fix must cut
  bytes (e.g. build sqb broadcasts on-device during the idle start) or
  keep a spare ready workstream (x0+s0 early) for the out-of-order
  tile scheduler.

2x_1p custom-uop experiment (session 3, measured on HW, see dve2x.py
and the mb*.py probes for the harness):
- A hand-authored uops_2x program (DveOpSpec.uops_2x via
  lower(minn(Src0+Src1, Src0Hi+Src1Hi)) with perf_max=1) DOES engage:
  fp16 SBUF in0/in1 step-1 => 2 elem/cycle/lane, confirmed 1135ns for
  2048 elems. perf_max must be passed at InstCustomDveAnt
  construction (byte-36[6]); mutating the instruction after
  add_instruction does not stick (it stores a clone).
- The ELEMENTWISE datapath in 2x mode is exactly correct (pair-mins
  at even output positions, hi lanes fine, lower()'s inp[1..] lane
  layout works unchanged; no stock-position remap needed). BUT the
  out-stream path buys nothing: the op consumes W inputs in W/2
  cycles and emits W/2 pair-mins, so 2x-op + 1x fold over the
  stride-2 evens = W/2 + W/2 = W cycles, identical to one direct 1x
  ADD_MIN (measured on HW: staged c1/c2 via ScalarE copy + 2x + fold
  ran 104.9us vs 101.8us direct - reverted). Any pairwise tree at 2x
  totals ~W cycles for the same reason. Only the broken fused ACCUM
  (W/2 cycles total, no second touch) would beat the 1x roofline.
- The ACCUM block in 2x mode is broken: accum_out returns a packed
  fp16 pair [v|v] of a wrong value, identical across init-wiring
  variants (C1 at stage3, stage0, stock lane remap). Blocks the
  dual-engine plan (ScalarE psum->fp16 copy + DVE 2x fused add-min
  accum ~= 50us steady-state). A fix would need the real accum-block
  RTL semantics for packed modes.
- Stock TT(min/add) fp16 SBUF = 2x_1p (confirmed ~0.55ns/elem);
  TT fp32 = 1x; tensor_reduce/TTR = 1x always; ScalarE activation
  Copy psum[128,1024]->SBUF fp16 measured via (172+FD)/1.2GHz. A
  stock-only staged pipeline (TT add + TT min tree) loses: the +sq_j
  pass eats the 2x gain (no per-element bias path on ScalarE; bias AP
  is per-partition, and Copy rejects AP bias - use Identity).
- gpsimd.tensor_tensor(min) fails to compile through the axon
  backend (opaque CallFunctionObjArgs error) - not usable.
"""

import os
import sys

sys.path.insert(0, "/opt/trn_rl_repo")

import numpy as np

import concourse.bacc as bacc
import concourse.mybir as mybir
import concourse.tile as tile
from concourse import bass_utils

f32 = mybir.dt.float32
f16 = mybir.dt.float16
Alu = mybir.AluOpType
Act = mybir.ActivationFunctionType

BIGB = 60000.0
TAU = 1.0
MARGIN = 0.3
PAD = 1024  # rotation padding so every device slice is contiguous
EQW = 512  # eq-masked tail region per tile (window is its last 256)
WINW = 256  # true positive window width
ZOFF = EQW - 192  # label zone starts at cR - ZOFF (sweep tail alignment)
NPAIR = 0  # chunk pairs per tile routed via scalar-engine copy + PAIR_MIN

TRACE = False
LAST_RESULT = None

_NC_CACHE = {}
_OPS_REGISTERED = {}


def _register_ops():
    """Fused DVE ops: cand = in0 + B*[in1 == s0], reduced with MIN
    (hardest negative) or MAX (hardest positive), accumulator seeded from s1
    for cross-chunk chaining."""
    if _OPS_REGISTERED:
        return _OPS_REGISTERED
    import concourse.dve_ops as dve_ops
    from concourse.dve_ops import OPS, DveOp, get_dve_sub_opcode
    from concourse.dve_spec import C0, C1, C2, Spec, Src0, Src1, eq, lower
    from concourse.dve_spec import AluOp as SAlu
    from concourse.dve_uop import DveOpSpec

    def make(name, accum_op, np_red):
        body = Src0 + eq(Src1, C0) * C2

        def ref(in0, in1, s0, s1, imm2):
            cand = (
                in0.astype(np.float32)
                + (in1.astype(np.float32) == s0) * np.float32(imm2)
            ).astype(np.float32)
            red = np_red(cand.reshape(cand.shape[0], -1), axis=-1, keepdims=True)
            seed = np.broadcast_to(np.asarray(s1, np.float32).reshape(-1, 1), red.shape)
            red = np_red(np.concatenate([red, seed], axis=1), axis=-1, keepdims=True)
            return cand, red

        spec = Spec(body=body, accum=accum_op, accum_init=C1, reference=ref)
        op = DveOp(name, spec, subdim=False, uops_sha={})
        OPS.append(op)
        dve_ops._SUB_OPCODE_FOR_NAME[name] = (
            dve_ops._CUSTOM_DVE_ROW_BASE + len(OPS) - 1
        )
        dve_ops.CUSTOM_DVE_SPECS[name] = spec
        assert dve_ops._SUB_OPCODE_FOR_NAME[name] < 0x20
        shas = {}
        for ver in ("v3", "v4"):
            try:
                dos = DveOpSpec(
                    name=name,
                    opcode=get_dve_sub_opcode(name),
                    uops=lower(spec, ver=ver),
                    rd1_en=True,
                )
                shas[ver] = dos.sha(ver)
            except Exception:
                pass
        object.__setattr__(op, "uops_sha", shas)
        return op

    def make_addmin(name):
        body = Src0 + Src1

        def ref(in0, in1, s0, s1, imm2):
            cand = (in0.astype(np.float32) + in1.astype(np.float32)).astype(
                np.float32
            )
            red = np.min(cand.reshape(cand.shape[0], -1), axis=-1, keepdims=True)
            seed = np.broadcast_to(np.asarray(s1, np.float32).reshape(-1, 1), red.shape)
            red = np.min(np.concatenate([red, seed], axis=1), axis=-1, keepdims=True)
            return cand, red

        spec = Spec(body=body, accum=SAlu.MIN, accum_init=C1, reference=ref)
        op = DveOp(name, spec, subdim=False, uops_sha={})
        OPS.append(op)
        dve_ops._SUB_OPCODE_FOR_NAME[name] = (
            dve_ops._CUSTOM_DVE_ROW_BASE + len(OPS) - 1
        )
        dve_ops.CUSTOM_DVE_SPECS[name] = spec
        assert dve_ops._SUB_OPCODE_FOR_NAME[name] < 0x20
        shas = {}
        for ver in ("v3", "v4"):
            try:
                dos = DveOpSpec(
                    name=name,
                    opcode=get_dve_sub_opcode(name),
                    uops=lower(spec, ver=ver),
                    rd1_en=True,
                )
                shas[ver] = dos.sha(ver)
            except Exception:
                pass
        object.__setattr__(op, "uops_sha", shas)
        return op

    def make_pairmin(name):
        from concourse.dve_spec import minn

        body = minn(Src0, Src1)

        def ref(in0, in1, s0, s1, imm2):
            cand = np.minimum(
                in0.astype(np.float32), in1.astype(np.float32)
            ).astype(np.float32)
            red = np.min(cand.reshape(cand.shape[0], -1), axis=-1, keepdims=True)
            seed = np.broadcast_to(np.asarray(s1, np.float32).reshape(-1, 1), red.shape)
            red = np.min(np.concatenate([red, seed], axis=1), axis=-1, keepdims=True)
            return cand, red

        spec = Spec(body=body, accum=SAlu.MIN, accum_init=C1, reference=ref)
        op = DveOp(name, spec, subdim=False, uops_sha={})
        OPS.append(op)
        dve_ops._SUB_OPCODE_FOR_NAME[name] = (
            dve_ops._CUSTOM_DVE_ROW_BASE + len(OPS) - 1
        )
        dve_ops.CUSTOM_DVE_SPECS[name] = spec
        assert dve_ops._SUB_OPCODE_FOR_NAME[name] < 0x20
        shas = {}
        for ver in ("v3", "v4"):
            try:
                dos = DveOpSpec(
                    name=name,
                    opcode=get_dve_sub_opcode(name),
                    uops=lower(spec, ver=ver),
                    rd1_en=True,
                )
                shas[ver] = dos.sha(ver)
            except Exception:
                pass
        object.__setattr__(op, "uops_sha", shas)
        return op

    _OPS_REGISTERED["min"] = make("BHTL_CAND_MIN", SAlu.MIN, np.min)
    _OPS_REGISTERED["max"] = make("BHTL_CAND_MAX", SAlu.MAX, np.max)
    _OPS_REGISTERED["addmin"] = make_addmin("BHTL_ADD_MIN")
    _OPS_REGISTERED["pairmin"] = make_pairmin("BHTL_PAIR_MIN")

    # ADD_MIN with a hand-authored 2x_1p program. The 2x ACCUM is broken in
    # HW (see docstring) but the elementwise OUT stream is exact: pair-mins
    # of (in0+in1) land at the EVEN positions of `out`. Used as a 2x
    # first-pass; a 1x ADD_MIN over the stride-2 evens finishes the chunk.
    from concourse.dve_ops import _COMPILE_CACHE
    from concourse.dve_spec import minn, Leaf
    from concourse.dve_uop import InpSel

    def make_addmin2x(name):
        body1 = Src0 + Src1
        Src0Hi = Leaf(InpSel.SRC_0_HI)
        Src1Hi = Leaf(InpSel.SRC_1_HI)
        body2 = minn(Src0 + Src1, Src0Hi + Src1Hi)

        def ref(in0, in1, s0, s1, imm2):
            cand = (in0.astype(np.float32) + in1.astype(np.float32)).astype(
                np.float32
            )
            red = np.min(cand.reshape(cand.shape[0], -1), axis=-1, keepdims=True)
            seed = np.broadcast_to(
                np.asarray(s1, np.float32).reshape(-1, 1), red.shape
            )
            red = np.min(np.concatenate([red, seed], axis=1), axis=-1, keepdims=True)
            return cand, red

        spec1 = Spec(body=body1, accum=SAlu.MIN, accum_init=C1, reference=ref)
        spec2 = Spec(body=body2, accum=SAlu.MIN, accum_init=C1, reference=ref)
        op = DveOp(name, spec1, subdim=False, uops_sha={})
        OPS.append(op)
        dve_ops._SUB_OPCODE_FOR_NAME[name] = (
            dve_ops._CUSTOM_DVE_ROW_BASE + len(OPS) - 1
        )
        dve_ops.CUSTOM_DVE_SPECS[name] = spec1
        assert dve_ops._SUB_OPCODE_FOR_NAME[name] < 0x20
        u1 = lower(spec1, ver="v3")
        u2 = lower(spec2, ver="v3")
        assert len(u1) == len(u2)
        dos = DveOpSpec(
            name=name,
            opcode=get_dve_sub_opcode(name),
            uops=u1,
            uops_2x=u2,
            perf_max=1,
            rd1_en=True,
        )
        object.__setattr__(op, "uops_sha", {"v3": dos.sha("v3")})
        _COMPILE_CACHE[(name, "v3")] = dos
        return op

    _OPS_REGISTERED["addmin2x"] = make_addmin2x("BHTL_ADD_MIN2X")
    return _OPS_REGISTERED


def _custom_dve_2x(vec, op, out, in0, in1, s0, s1, imm2, accum_out):
    """_custom_dve clone passing perf_max=1 at construction (byte-36[6];
    post-emission mutation does not stick - add_instruction clones)."""
    from concourse import bass_isa
    from concourse.dve_ops import get_dve_sub_opcode

    bass = vec.bass
    if op.name not in bass.m.ant_custom_dve_ops:
        bass.m.ant_custom_dve_ops = sorted({*bass.m.ant_custom_dve_ops, op.name})
    shape = bass_isa.CustomDveShape.TTSS
    isa_opcode = bass.isa.Opcode[
        f"NEURON_ISA_TPB_OPCODE_CUSTOM_DVE_ANT_{shape.slot()}"
    ].value

    def lsc(v):
        if isinstance(v, (int, float)):
            return mybir.ImmediateValue(dtype=mybir.dt.float32, value=float(v))
        return vec.lower_ap(v, for_isa=True)

    ins = [
        vec.lower_ap(in0, for_isa=True, opt=True),
        vec.lower_ap(in1, for_isa=True, opt=True),
        lsc(s0),
        lsc(s1),
    ]
    outs = [
        vec.lower_ap(out, for_isa=True, opt=True),
        vec.lower_ap(accum_out, for_isa=True),
    ]
    return vec.add_instruction(
        bass_isa.InstCustomDveAnt(
            name=bass.get_next_instruction_name(),
            op_name=op.name,
            rd1_en=True,
            subdim=0,
            imm2=imm2,
            shape=shape,
            row=get_dve_sub_opcode(op.name),
            isa_opcode=isa_opcode,
            ins=ins,
            outs=outs,
            perf_max=1,
        )
    )


def build_nc(N, M):
    R = N // M  # rows per core
    T = R // 128  # 128-row tiles per core
    NP = N + PAD
    ZW = 128 * (T - 1) + EQW  # label zone width (1408 for T=8)

    ops = _register_ops()
    op_min, op_max = ops["min"], ops["max"]
    op_addmin = ops["addmin"]
    op_addmin2x = ops["addmin2x"]

    nc = bacc.Bacc("TRN2", target_bir_lowering=False, debug=False)

    xTrot_d = nc.dram_tensor("xTrot", [128, NP], f16, kind="ExternalInput")
    m2slab_d = nc.dram_tensor("m2slab", [128, R], f16, kind="ExternalInput")
    labz_d = nc.dram_tensor("labz", [1, ZW], f16, kind="ExternalInput")
    mylab_d = nc.dram_tensor("mylab", [128, T], f32, kind="ExternalInput")
    sqrot_d = nc.dram_tensor("sqrot", [1, NP], f16, kind="ExternalInput")
    out_d = nc.dram_tensor("out", [128, 2 * T], f32, kind="ExternalOutput")

    with tile.TileContext(nc) as tc:
        with tc.tile_pool(name="const", bufs=1) as cp:
            # v2.2 geometry: 4 chunks of 2048 per tile; the eq zone is the
            # tail 512 of chunk 3 (K=1 sq matmul accumulates there), which
            # drops the separate xt_eq slice and the separate eq psum pool.
            CW = 2048
            SW = CW + 128 * (T - 1)  # 2944: per-chunk-index slice width
            EO = 4 * CW - EQW  # 7680: eq zone offset in the sweep
            FW = 512  # tile-0 fast-start width

            m2a = cp.tile([128, 384], f16)  # weights for tiles 0-2
            m2rest = cp.tile([128, R - 384], f16)  # tiles 3-7, loaded later
            sqrow = cp.tile([1, NP], f16)
            xts = [cp.tile([128, SW], f16, name=f"xt{c}") for c in range(4)]
            sqbs = [cp.tile([128, SW], f16, name=f"sqb{c}") for c in range(4)]
            labz = cp.tile([128, ZW], f16)
            mylab = cp.tile([128, T], f32)

            # Chunk-index-major consumption (below) means each landed slice
            # unlocks ~17us of DVE work across all 8 tiles, so the two DMA
            # queues stay ahead after chunk 0. Order: fast-start pair first,
            # then x on sync / sqb on scalar in chunk order, chunk-0 split
            # so tile0's remainder starts early.
            # tile0 needs only m2a[0:128] for its first op; tiles 1-2's
            # weights follow the first x piece so they don't delay it
            nc.sync.dma_start(m2a[:, 0:128], m2slab_d.ap()[:, 0:128])
            first = True
            # chunk-0 slices in four pieces per queue (subtile deps are
            # tracked, so tile0's first ops start as soon as their piece
            # lands - no duplicate fast-start tiles needed)
            for lo, hi in ((0, FW), (FW, 1024), (1024, 2176)):
                nc.sync.dma_start(xts[0][:, lo:hi], xTrot_d.ap()[:, lo:hi])
                nc.scalar.dma_start(
                    sqbs[0][:, lo:hi],
                    sqrot_d.ap()[:, lo:hi].broadcast_to([128, hi - lo]),
                )
                if first:
                    nc.sync.dma_start(
                        m2a[:, 128:384], m2slab_d.ap()[:, 128:384]
                    )
                    first = False
            nc.sync.dma_start(xts[0][:, 2176:SW], xTrot_d.ap()[:, 2176:SW])
            nc.scalar.dma_start(
                sqbs[0][:, 2176:SW],
                sqrot_d.ap()[:, 2176:SW].broadcast_to([128, SW - 2176]),
            )
            nc.scalar.dma_start(m2rest[:], m2slab_d.ap()[:, 384:R])
            for c in range(1, 4):
                nc.sync.dma_start(
                    xts[c][:], xTrot_d.ap()[:, c * CW : c * CW + SW]
                )
                nc.scalar.dma_start(
                    sqbs[c][:],
                    sqrot_d.ap()[:, c * CW : c * CW + SW].broadcast_to([128, SW]),
                )
            nc.sync.dma_start(labz[:], labz_d.ap().broadcast_to([128, ZW]))
            nc.sync.dma_start(mylab[:], mylab_d.ap())
            nc.scalar.dma_start(sqrow[:], sqrot_d.ap())

            ones_row = cp.tile([1, 128], f16)
            nc.vector.memset(ones_row[:], 1.0)
            zeros256 = cp.tile([128, WINW], f16)
            nc.vector.memset(zeros256[:], 0.0)
            zeros1024 = cp.tile([128, CW // 2], f16)
            nc.vector.memset(zeros1024[:], 0.0)

            with (
                tc.tile_pool(name="psum", bufs=2, space="PSUM") as pp,
                tc.tile_pool(name="dum", bufs=2) as dp,
                tc.tile_pool(name="st", bufs=3) as stp,
                tc.tile_pool(name="acc", bufs=28) as acp,
            ):
                chains = {t: [None, None] for t in range(T)}
                nch = {t: 0 for t in range(T)}  # accum-op parity per tile

                def addmin(t, ps_ap, sq_ap, wd, tag):
                    dum = dp.tile([128, wd], f16, tag=tag)
                    nacc = acp.tile([128, 1], f32, tag="acc")
                    k = nch[t] % 2
                    nch[t] += 1
                    nc.vector._custom_dve(
                        op_addmin,
                        out=dum[:],
                        in0=ps_ap,
                        in1=sq_ap,
                        s0=0.0,
                        s1=(1e30 if chains[t][k] is None else chains[t][k][:]),
                        imm2=0.0,
                        accum_out=nacc[:],
                    )
                    chains[t][k] = nacc

                for c in range(4):
                    for t in range(T):
                        base = 128 * t
                        w = (
                            m2a[:, base : base + 128]
                            if t < 3
                            else m2rest[:, base - 384 : base - 256]
                        )
                        if c == 0 and t == 0:
                            # fast-start: narrow first chunk, then the
                            # remainder in two pieces matching the DMA splits
                            ps = pp.tile([128, FW], f32, tag="ps")
                            nc.tensor.matmul(
                                ps[:], w, xts[0][:, 0:FW], start=True, stop=True
                            )
                            addmin(0, ps[:], sqbs[0][:, 0:FW], FW, "dumf")
                            for lo, hi in ((FW, 1024), (1024, CW)):
                                ps = pp.tile([128, hi - lo], f32, tag="ps")
                                for q in range((hi - lo) // 512):
                                    o = lo + 512 * q
                                    nc.tensor.matmul(
                                        ps[:, 512 * q : 512 * q + 512],
                                        w,
                                        xts[0][:, o : o + 512],
                                        start=True,
                                        stop=True,
                                    )
                                addmin(
                                    0, ps[:], sqbs[0][:, lo:hi], hi - lo, "dum"
                                )
                            continue
                        ps = pp.tile([128, CW], f32, tag="ps")
                        nq = 4 if c < 3 else 3
                        for q in range(nq):
                            o = base + 512 * q
                            nc.tensor.matmul(
                                ps[:, 512 * q : 512 * q + 512],
                                w,
                                xts[c][:, o : o + 512],
                                start=True,
                                stop=True,
                            )
                        if c < 3:
                            addmin(
                                t, ps[:], sqbs[c][:, base : base + CW], CW, "dum"
                            )
                            continue
                        # chunk 3: plain 1536 + eq tail 512 (K=1 sq matmul)
                        nc.tensor.matmul(
                            ps[:, CW - EQW : CW],
                            ones_row[:],
                            sqrow[0:1, EO + base : EO + base + EQW],
                            start=True,
                            stop=False,
                        )
                        nc.tensor.matmul(
                            ps[:, CW - EQW : CW],
                            w,
                            xts[3][:, base + CW - EQW : base + CW],
                            start=False,
                            stop=True,
                        )
                        pw = CW - EQW  # 1536
                        # stage the eq tail to SBUF fp16 on the (idle) scalar
                        # engine so the big psum buf frees right after the
                        # plain reduction; otherwise the CAND ops hold it and
                        # the PE pipeline bubbles ~1.5us/tile (measured).
                        eqst = dp.tile([128, EQW], f16, tag="eqst")
                        nc.scalar.activation(eqst[:], ps[:, pw:CW], Act.Copy)
                        addmin(t, ps[:, 0:pw], sqbs[3][:, base : base + pw], pw, "dum")
                        # prewindow half of the eq zone: all different-label
                        addmin(t, eqst[:, 0:WINW], zeros256[:], WINW, "dz")
                        mg = acp.tile([128, 1], f32, tag="mg")
                        nc.vector.tensor_tensor(
                            mg[:], chains[t][0][:], chains[t][1][:], op=Alu.min
                        )
                        # eq-masked min over the true window -> hardest negative
                        ed = dp.tile([128, WINW], f16, tag="eqd")
                        nm = acp.tile([128, 1], f32, tag="nm")
                        nc.vector._custom_dve(
                            op_min,
                            out=ed[:],
                            in0=eqst[:, WINW:EQW],
                            in1=labz[:, base + WINW : base + EQW],
                            s0=mylab[:, t : t + 1],
                            s1=mg[:],
                            imm2=BIGB,
                            accum_out=nm[:],
                        )
                        nc.scalar.dma_start(out_d.ap()[:, t : t + 1], nm[:])
                        # eq-masked max over the true window -> hardest positive
                        ed2 = dp.tile([128, WINW], f16, tag="eqd2")
                        pm = acp.tile([128, 1], f32, tag="pm")
                        nc.vector._custom_dve(
                            op_max,
                            out=ed2[:],
                            in0=eqst[:, WINW:EQW],
                            in1=labz[:, base + WINW : base + EQW],
                            s0=mylab[:, t : t + 1],
                            s1=-1e30,
                            imm2=BIGB,
                            accum_out=pm[:],
                        )
                        nc.scalar.dma_start(out_d.ap()[:, T + t : T + t + 1], pm[:])

    nc.compile()
    return nc


def _prep_inputs(x, labels, M):
    """Sort rows by label; build per-core pre-rotated, padded inputs.
    Validates that every row's label group falls inside the per-tile
    window [128*floor(r/128) - 64, 128*floor(r/128) + 192)."""
    N, D = x.shape
    R = N // M
    T = R // 128
    labels = np.asarray(labels)
    perm = np.argsort(labels, kind="stable")
    xs = np.ascontiguousarray(x[perm])
    ls = labels[perm]
    sq = (xs.astype(np.float64) ** 2).sum(1)

    # group bounds per row
    bounds = np.flatnonzero(np.diff(ls)) + 1
    starts = np.concatenate([[0], bounds])
    ends = np.concatenate([bounds, [N]])
    sizes = ends - starts
    first = np.repeat(starts, sizes)
    last = np.repeat(ends - 1, sizes)
    tf = (np.arange(N) // 128) * 128
    windows_ok = bool((first >= tf - 64).all() and (last <= tf + 191).all())

    xsT16 = np.ascontiguousarray(xs.T.astype(np.float16))  # [128, N]
    sq16 = sq.astype(np.float16)
    ls16 = ls.astype(np.float16)
    ZW = 128 * (T - 1) + EQW

    in_maps = []
    for c in range(M):
        rot0 = (c * R + 192) % N
        idx = (rot0 + np.arange(N + PAD)) % N
        zidx = (c * R - ZOFF + np.arange(ZW)) % N
        rows = c * R + np.arange(R)
        in_maps.append(
            {
                "xTrot": np.ascontiguousarray(xsT16[:, idx]),
                "m2slab": np.ascontiguousarray(
                    (-2.0 * xs[rows]).T.astype(np.float16)
                ),
                "labz": np.ascontiguousarray(ls16[zidx].reshape(1, ZW)),
                "mylab": np.ascontiguousarray(
                    ls[rows].astype(np.float32).reshape(T, 128).T
                ),
                "sqi": np.ascontiguousarray(
                    sq[rows].astype(np.float32).reshape(T, 128).T
                ),
                "sqrot": np.ascontiguousarray(sq16[idx].reshape(1, N + PAD)),
            }
        )
    return in_maps, windows_ok


def kernel(embeddings, labels):
    global LAST_RESULT
    x = np.asarray(embeddings, dtype=np.float32)
    lab = np.asarray(labels)
    N, D = x.shape
    M = 8
    assert D == 128 and N % (M * 128) == 0

    in_maps, windows_ok = _prep_inputs(x, lab, M)
    assert windows_ok, "label-group window invariant violated"
    key = (N, M)
    if key not in _NC_CACHE:
        _NC_CACHE[key] = build_nc(N, M)
    nc = _NC_CACHE[key]

    if TRACE:
        _install_ntff_hook()
    dev_maps = [{k: v for k, v in m.items() if k != "sqi"} for m in in_maps]
    res = bass_utils.run_bass_kernel_spmd(
        nc, dev_maps, core_ids=list(range(M)), trace=TRACE
    )
    LAST_RESULT = res

    R = N // M
    T = R // 128
    total = 0.0
    cnt = 0.0
    for c in range(M):
        o = res.results[c]["out"].astype(np.float64)
        negmin = o[:, 0:T]
        posmax = o[:, T : 2 * T]
        sqi = in_maps[c]["sqi"].astype(np.float64)
        hp2 = posmax - BIGB + sqi
        hn2 = negmin + sqi
        valid = (hp2 > TAU) & (hn2 < BIGB / 2.0)
        hp = np.sqrt(np.maximum(hp2, 0.0))
        hn = np.sqrt(np.maximum(hn2, 0.0))
        pr = np.maximum(hp + MARGIN - hn, 0.0) * valid
        total += pr.sum()
        cnt += valid.sum()
    loss = total / max(cnt, 1.0) if cnt > 0 else 0.0
    return np.float32(loss)


def _install_ntff_hook():
    """The container's antenv stub lacks axon_hooks; provide it so
    run_bass_kernel_spmd(trace=True) can capture NTFF profiles."""
    import contextlib
    import ctypes
    import types

    try:
        from antenv.axon_hooks import get_axon_ntff_profile_hook  # noqa: F401

        return
    except ImportError:
        pass
    import antenv

    mod = types.ModuleType("antenv.axon_hooks")
    _h = {"h": None}
    mod.set_axon_ntff_profile_hook = lambda h: _h.__setitem__("h", h)
    mod.get_axon_ntff_profile_hook = lambda: _h["h"]
    sys.modules["antenv.axon_hooks"] = mod
    antenv.axon_hooks = mod

    so_path = "/opt/axon/libaxon_pjrt.so"
    if not os.path.exists(so_path):
        return
    lib = ctypes.CDLL(so_path)
    if not hasattr(lib, "axon_start_nrt_profile"):
        return
    lib.axon_start_nrt_profile.argtypes = [
        ctypes.POINTER(ctypes.c_int64),
        ctypes.c_size_t,
    ]
    lib.axon_start_nrt_profile.restype = ctypes.c_int64
    lib.axon_stop_nrt_profile.argtypes = [ctypes.c_char_p]
    lib.axon_stop_nrt_profile.restype = ctypes.c_int64

    @contextlib.contextmanager
    def _hook(output_dir, device_ids):
        import jax

        jax.devices()
        if device_ids:
            ids = (ctypes.c_int64 * len(device_ids))(*device_ids)
            rc = lib.axon_start_nrt_profile(ids, len(device_ids))
        else:
            rc = lib.axon_start_nrt_profile(None, 0)
        if rc != 0:
            raise RuntimeError(f"axon_start_nrt_profile rc={rc}")
        try:
            yield
        finally:
            n = lib.axon_stop_nrt_profile(str(output_dir).encode())
            print(f"profile: {n} file(s) written to {output_dir}", file=sys.stderr)

    mod.set_axon_ntff_profile_hook(_hook)



# revision 25
# speedup vs baseline: 1.0712x; 1.0712x over previous
"""BatchHardTripletLoss on 8 Trainium2 NeuronCores (Bass/Tile), v2.2.

Sharding: embeddings row-sharded 8 ways; each core computes its
[1024, 8192] slab of psum[i,j] = -2*x_i.x_j with fp16 matmuls and
reduces it on-device; per-row (negmin, posmax) go back to the host,
which finishes sq_i + sqrt + relu + mean (O(N) work).

Layout: rows are pre-sorted by label on host. Each core's column
stream is rotated PER TILE (host pads the rotated arrays by 1024 cols
so every device slice is contiguous): tile t reads columns starting at
global col cR + 128t + 192, which lands the tile's same-label window
in the LAST 512 columns of its 8192-col sweep. Everything before that
is guaranteed different-label, so the hardest-negative reduction runs
as a fused custom DVE op ADD_MIN (cand = psum + sq_j broadcast, with a
chained min accumulator). The eq zone (final 512 cols, sq_j via a K=1
accumulate matmul) uses the eq-masked custom ops CAND_MIN/CAND_MAX;
the self-pair anchors no-positive detection via TAU.

v2.2 structure (106.9us -> 101.8us measured):
- 4 chunks of 2048 per tile (psum = 2 bufs x [128,2048] = all 8
  banks); the eq zone rides in the tail 512 of chunk 3, dropping the
  separate eq psum pool, the xt_eq slice, and one DVE op per tile.
- CHUNK-INDEX-MAJOR sweep (all 8 tiles' chunk c, then c+1): each
  landed input slice (x_c on sync queue, sqb_c broadcast on scalar
  queue, ~750KB each at ~110-125KB/us/queue) unlocks ~17us of DVE
  work, so after chunk 0 the two DMA queues stay ahead and the DVE
  runs gap-free at its 1 elem/cycle/lane roofline. Tile-major order
  (v2) instead stalled ~2us per early chunk and started 6us later.
  Per-tile accumulators live across sweeps (chains dict, acc pool).
- Tile 0's chunk 0 is split 512+512+1024 with matching split DMA
  pieces (subtile deps are tracked, so no duplicate fast-start tiles
  are needed), and the weight slab loads as [0:384] early on sync
  (tiles 0-2) + [384:1024] late on scalar: the first reduction starts
  ~13us and tile 1 unblocks ~2us earlier (98.5-99.9us measured).
- The eq tail is staged psum->SBUF fp16 by the otherwise-idle scalar
  engine so the big psum buf frees right after the plain 1536-wide
  reduction; without this the CAND ops hold the buf and the PE
  pipeline bubbles ~3.4us per 2 tiles in the chunk-3 sweep (measured).
- Remaining non-compute time: ~8us preamble+DMA-spinup to first op,
  ~4us of supply-rate-bound lead-in gaps, ~3us postamble ceremony.
  DVE busy ~81us (66us streaming + ~115ns/op overhead x 49 ops +
  CAND/merge extras) is the roofline wall; see the 2x notes below for
  why a second reduce lane is blocked.

Notes from HW profiling (all verified on device this series):
- tensor_tensor_reduce (stock DVE) with a PSUM input crashes the
  device even with one PSUM operand; the compiler verifier separately
  rejects two PSUM reads (NCC_IBVF027). Custom DVE ops with in0=psum,
  in1=SBUF are the proven pattern, at 1 elem/cycle/lane (0.96 GHz).
- Matmuls stream at ~295ns/512 cols when the PE is saturated but
  ~600ns in this DVE-bound kernel (HAM throttle never lifts), which
  is why scalar-engine-staged pair-min (needs K=1 sq matmuls) lost in
  three measured attempts.
- gpsimd tensor_reduce supports only partition-axis (C/XYZWC) - no
  free-axis reduction offload; gpsimd-issued DMAs use slow software
  DGE. The activation engine copies psum->SBUF fine but has no min.
- Startup is ~6us fixed preamble + ~2.5us DMA-queue spin-up before
  any transfer; whole-tile DMA dependencies (not bandwidth) gated the
  first reduction until inputs were split per chunk index. v2.1: the
  first ADD_MIN additionally waits on queue predecessors (sqrow +
  m2slab + full xts[0] ~ 896KB on one queue => first op at ~18.9us).
  A fast-start restructure (narrow 512-col tile-0 chunk, dedicated
  m2a/xfirst/sqbfirst tiles loaded first) DID pull the first op to
  12.6us, but serializing all x-slices on sync / sqb-slices on scalar
  starved later chunks (7.3us stall at tile0-chunk3): net 111.6us vs
  107.1us, so it was reverted. Only SP and Activation can initiate
  HWDGE DMA (vector cannot). The early phase is DMA-rate-bound
  (~110-125KB/us per queue, ~6.7MB of inputs); a real fix must cut
  bytes (e.g. build sqb broadcasts on-device during the idle start) or
  keep a spare ready workstream (x0+s0 early) for the out-of-order
  tile scheduler.

2x_1p custom-uop experiment (session 3, measured on HW, see dve2x.py
and the mb*.py probes for the harness):
- A hand-authored uops_2x program (DveOpSpec.uops_2x via
  lower(minn(Src0+Src1, Src0Hi+Src1Hi)) with perf_max=1) DOES engage:
  fp16 SBUF in0/in1 step-1 => 2 elem/cycle/lane, confirmed 1135ns for
  2048 elems. perf_max must be passed at InstCustomDveAnt
  construction (byte-36[6]); mutating the instruction after
  add_instruction does not stick (it stores a clone).
- The ELEMENTWISE datapath in 2x mode is exactly correct (pair-mins
  at even output positions, hi lanes fine, lower()'s inp[1..] lane
  layout works unchanged; no stock-position remap needed). BUT the
  out-stream path buys nothing: the op consumes W inputs in W/2
  cycles and emits W/2 pair-mins, so 2x-op + 1x fold over the
  stride-2 evens = W/2 + W/2 = W cycles, identical to one direct 1x
  ADD_MIN (measured on HW: staged c1/c2 via ScalarE copy + 2x + fold
  ran 104.9us vs 101.8us direct - reverted). Any pairwise tree at 2x
  totals ~W cycles for the same reason. Only the broken fused ACCUM
  (W/2 cycles total, no second touch) would beat the 1x roofline.
- The ACCUM block in 2x mode is broken: accum_out returns a packed
  fp16 pair [v|v] of a wrong value, identical across init-wiring
  variants (C1 at stage3, stage0, stock lane remap). Blocks the
  dual-engine plan (ScalarE psum->fp16 copy + DVE 2x fused add-min
  accum ~= 50us steady-state). A fix would need the real accum-block
  RTL semantics for packed modes.
- Stock TT(min/add) fp16 SBUF = 2x_1p (confirmed ~0.55ns/elem);
  TT fp32 = 1x; tensor_reduce/TTR = 1x always; ScalarE activation
  Copy psum[128,1024]->SBUF fp16 measured via (172+FD)/1.2GHz. A
  stock-only staged pipeline (TT add + TT min tree) loses: the +sq_j
  pass eats the 2x gain (no per-element bias path on ScalarE; bias AP
  is per-partition, and Copy rejects AP bias - use Identity).
- gpsimd.tensor_tensor(min) fails to compile through the axon
  backend (opaque CallFunctionObjArgs error) - not usable.
"""

import os
import sys

sys.path.insert(0, "/opt/trn_rl_repo")

import numpy as np

import concourse.bacc as bacc
import concourse.mybir as mybir
import concourse.tile as tile
from concourse import bass_utils

f32 = mybir.dt.float32
f16 = mybir.dt.float16
Alu = mybir.AluOpType
Act = mybir.ActivationFunctionType

BIGB = 60000.0
TAU = 1.0
MARGIN = 0.3
PAD = 1024  # rotation padding so every device slice is contiguous
EQW = 512  # eq-masked tail region per tile (window is its last 256)
WINW = 256  # true positive window width
ZOFF = EQW - 192  # label zone starts at cR - ZOFF (sweep tail alignment)
NPAIR = 0  # chunk pairs per tile routed via scalar-engine copy + PAIR_MIN

TRACE = False
LAST_RESULT = None

_NC_CACHE = {}
_OPS_REGISTERED = {}


def _register_ops():
    """Fused DVE ops: cand = in0 + B*[in1 == s0], reduced with MIN
    (hardest negative) or MAX (hardest positive), accumulator seeded from s1
    for cross-chunk chaining."""
    if _OPS_REGISTERED:
        return _OPS_REGISTERED
    import concourse.dve_ops as dve_ops
    from concourse.dve_ops import OPS, DveOp, get_dve_sub_opcode
    from concourse.dve_spec import C0, C1, C2, Spec, Src0, Src1, eq, lower
    from concourse.dve_spec import AluOp as SAlu
    from concourse.dve_uop import DveOpSpec

    def make(name, accum_op, np_red):
        body = Src0 + eq(Src1, C0) * C2

        def ref(in0, in1, s0, s1, imm2):
            cand = (
                in0.astype(np.float32)
                + (in1.astype(np.float32) == s0) * np.float32(imm2)
            ).astype(np.float32)
            red = np_red(cand.reshape(cand.shape[0], -1), axis=-1, keepdims=True)
            seed = np.broadcast_to(np.asarray(s1, np.float32).reshape(-1, 1), red.shape)
            red = np_red(np.concatenate([red, seed], axis=1), axis=-1, keepdims=True)
            return cand, red

        spec = Spec(body=body, accum=accum_op, accum_init=C1, reference=ref)
        op = DveOp(name, spec, subdim=False, uops_sha={})
        OPS.append(op)
        dve_ops._SUB_OPCODE_FOR_NAME[name] = (
            dve_ops._CUSTOM_DVE_ROW_BASE + len(OPS) - 1
        )
        dve_ops.CUSTOM_DVE_SPECS[name] = spec
        assert dve_ops._SUB_OPCODE_FOR_NAME[name] < 0x20
        shas = {}
        for ver in ("v3", "v4"):
            try:
                dos = DveOpSpec(
                    name=name,
                    opcode=get_dve_sub_opcode(name),
                    uops=lower(spec, ver=ver),
                    rd1_en=True,
                )
                shas[ver] = dos.sha(ver)
            except Exception:
                pass
        object.__setattr__(op, "uops_sha", shas)
        return op

    def make_addmin(name):
        body = Src0 + Src1

        def ref(in0, in1, s0, s1, imm2):
            cand = (in0.astype(np.float32) + in1.astype(np.float32)).astype(
                np.float32
            )
            red = np.min(cand.reshape(cand.shape[0], -1), axis=-1, keepdims=True)
            seed = np.broadcast_to(np.asarray(s1, np.float32).reshape(-1, 1), red.shape)
            red = np.min(np.concatenate([red, seed], axis=1), axis=-1, keepdims=True)
            return cand, red

        spec = Spec(body=body, accum=SAlu.MIN, accum_init=C1, reference=ref)
        op = DveOp(name, spec, subdim=False, uops_sha={})
        OPS.append(op)
        dve_ops._SUB_OPCODE_FOR_NAME[name] = (
            dve_ops._CUSTOM_DVE_ROW_BASE + len(OPS) - 1
        )
        dve_ops.CUSTOM_DVE_SPECS[name] = spec
        assert dve_ops._SUB_OPCODE_FOR_NAME[name] < 0x20
        shas = {}
        for ver in ("v3", "v4"):
            try:
                dos = DveOpSpec(
                    name=name,
                    opcode=get_dve_sub_opcode(name),
                    uops=lower(spec, ver=ver),
                    rd1_en=True,
                )
                shas[ver] = dos.sha(ver)
            except Exception:
                pass
        object.__setattr__(op, "uops_sha", shas)
        return op

    def make_pairmin(name):
        from concourse.dve_spec import minn

        body = minn(Src0, Src1)

        def ref(in0, in1, s0, s1, imm2):
            cand = np.minimum(
                in0.astype(np.float32), in1.astype(np.float32)
            ).astype(np.float32)
            red = np.min(cand.reshape(cand.shape[0], -1), axis=-1, keepdims=True)
            seed = np.broadcast_to(np.asarray(s1, np.float32).reshape(-1, 1), red.shape)
            red = np.min(np.concatenate([red, seed], axis=1), axis=-1, keepdims=True)
            return cand, red

        spec = Spec(body=body, accum=SAlu.MIN, accum_init=C1, reference=ref)
        op = DveOp(name, spec, subdim=False, uops_sha={})
        OPS.append(op)
        dve_ops._SUB_OPCODE_FOR_NAME[name] = (
            dve_ops._CUSTOM_DVE_ROW_BASE + len(OPS) - 1
        )
        dve_ops.CUSTOM_DVE_SPECS[name] = spec
        assert dve_ops._SUB_OPCODE_FOR_NAME[name] < 0x20
        shas = {}
        for ver in ("v3", "v4"):
            try:
                dos = DveOpSpec(
                    name=name,
                    opcode=get_dve_sub_opcode(name),
                    uops=lower(spec, ver=ver),
                    rd1_en=True,
                )
                shas[ver] = dos.sha(ver)
            except Exception:
                pass
        object.__setattr__(op, "uops_sha", shas)
        return op

    _OPS_REGISTERED["min"] = make("BHTL_CAND_MIN", SAlu.MIN, np.min)
    _OPS_REGISTERED["max"] = make("BHTL_CAND_MAX", SAlu.MAX, np.max)
    _OPS_REGISTERED["addmin"] = make_addmin("BHTL_ADD_MIN")
    _OPS_REGISTERED["pairmin"] = make_pairmin("BHTL_PAIR_MIN")

    # ADD_MIN with a hand-authored 2x_1p program. The 2x ACCUM is broken in
    # HW (see docstring) but the elementwise OUT stream is exact: pair-mins
    # of (in0+in1) land at the EVEN positions of `out`. Used as a 2x
    # first-pass; a 1x ADD_MIN over the stride-2 evens finishes the chunk.
    from concourse.dve_ops import _COMPILE_CACHE
    from concourse.dve_spec import minn, Leaf
    from concourse.dve_uop import InpSel

    def make_addmin2x(name):
        body1 = Src0 + Src1
        Src0Hi = Leaf(InpSel.SRC_0_HI)
        Src1Hi = Leaf(InpSel.SRC_1_HI)
        body2 = minn(Src0 + Src1, Src0Hi + Src1Hi)

        def ref(in0, in1, s0, s1, imm2):
            cand = (in0.astype(np.float32) + in1.astype(np.float32)).astype(
                np.float32
            )
            red = np.min(cand.reshape(cand.shape[0], -1), axis=-1, keepdims=True)
            seed = np.broadcast_to(
                np.asarray(s1, np.float32).reshape(-1, 1), red.shape
            )
            red = np.min(np.concatenate([red, seed], axis=1), axis=-1, keepdims=True)
            return cand, red

        spec1 = Spec(body=body1, accum=SAlu.MIN, accum_init=C1, reference=ref)
        spec2 = Spec(body=body2, accum=SAlu.MIN, accum_init=C1, reference=ref)
        op = DveOp(name, spec1, subdim=False, uops_sha={})
        OPS.append(op)
        dve_ops._SUB_OPCODE_FOR_NAME[name] = (
            dve_ops._CUSTOM_DVE_ROW_BASE + len(OPS) - 1
        )
        dve_ops.CUSTOM_DVE_SPECS[name] = spec1
        assert dve_ops._SUB_OPCODE_FOR_NAME[name] < 0x20
        u1 = lower(spec1, ver="v3")
        u2 = lower(spec2, ver="v3")
        assert len(u1) == len(u2)
        dos = DveOpSpec(
            name=name,
            opcode=get_dve_sub_opcode(name),
            uops=u1,
            uops_2x=u2,
            perf_max=1,
            rd1_en=True,
        )
        object.__setattr__(op, "uops_sha", {"v3": dos.sha("v3")})
        _COMPILE_CACHE[(name, "v3")] = dos
        return op

    _OPS_REGISTERED["addmin2x"] = make_addmin2x("BHTL_ADD_MIN2X")
    return _OPS_REGISTERED


def _custom_dve_2x(vec, op, out, in0, in1, s0, s1, imm2, accum_out):
    """_custom_dve clone passing perf_max=1 at construction (byte-36[6];
    post-emission mutation does not stick - add_instruction clones)."""
    from concourse import bass_isa
    from concourse.dve_ops import get_dve_sub_opcode

    bass = vec.bass
    if op.name not in bass.m.ant_custom_dve_ops:
        bass.m.ant_custom_dve_ops = sorted({*bass.m.ant_custom_dve_ops, op.name})
    shape = bass_isa.CustomDveShape.TTSS
    isa_opcode = bass.isa.Opcode[
        f"NEURON_ISA_TPB_OPCODE_CUSTOM_DVE_ANT_{shape.slot()}"
    ].value

    def lsc(v):
        if isinstance(v, (int, float)):
            return mybir.ImmediateValue(dtype=mybir.dt.float32, value=float(v))
        return vec.lower_ap(v, for_isa=True)

    ins = [
        vec.lower_ap(in0, for_isa=True, opt=True),
        vec.lower_ap(in1, for_isa=True, opt=True),
        lsc(s0),
        lsc(s1),
    ]
    outs = [
        vec.lower_ap(out, for_isa=True, opt=True),
        vec.lower_ap(accum_out, for_isa=True),
    ]
    return vec.add_instruction(
        bass_isa.InstCustomDveAnt(
            name=bass.get_next_instruction_name(),
            op_name=op.name,
            rd1_en=True,
            subdim=0,
            imm2=imm2,
            shape=shape,
            row=get_dve_sub_opcode(op.name),
            isa_opcode=isa_opcode,
            ins=ins,
            outs=outs,
            perf_max=1,
        )
    )


def build_nc(N, M):
    R = N // M  # rows per core
    T = R // 128  # 128-row tiles per core
    NP = N + PAD
    ZW = 128 * (T - 1) + EQW  # label zone width (1408 for T=8)

    ops = _register_ops()
    op_min, op_max = ops["min"], ops["max"]
    op_addmin = ops["addmin"]
    op_addmin2x = ops["addmin2x"]

    nc = bacc.Bacc("TRN2", target_bir_lowering=False, debug=False)

    xTrot_d = nc.dram_tensor("xTrot", [128, NP], f16, kind="ExternalInput")
    m2slab_d = nc.dram_tensor("m2slab", [128, R], f16, kind="ExternalInput")
    labz_d = nc.dram_tensor("labz", [1, ZW], f16, kind="ExternalInput")
    mylab_d = nc.dram_tensor("mylab", [128, T], f32, kind="ExternalInput")
    sqrot_d = nc.dram_tensor("sqrot", [1, NP], f16, kind="ExternalInput")
    out_d = nc.dram_tensor("out", [128, 2 * T], f32, kind="ExternalOutput")

    with tile.TileContext(nc) as tc:
        with tc.tile_pool(name="const", bufs=1) as cp:
            # v2.2 geometry: 4 chunks of 2048 per tile; the eq zone is the
            # tail 512 of chunk 3 (K=1 sq matmul accumulates there), which
            # drops the separate xt_eq slice and the separate eq psum pool.
            CW = 2048
            SW = CW + 128 * (T - 1)  # 2944: per-chunk-index slice width
            EO = 4 * CW - EQW  # 7680: eq zone offset in the sweep
            FW = 512  # tile-0 fast-start width

            m2a = cp.tile([128, 384], f16)  # weights for tiles 0-2
            m2rest = cp.tile([128, R - 384], f16)  # tiles 3-7, loaded later
            sqrow = cp.tile([1, NP], f16)
            xts = [cp.tile([128, SW], f16, name=f"xt{c}") for c in range(4)]
            sqbs = [cp.tile([128, SW], f16, name=f"sqb{c}") for c in range(4)]
            labz = cp.tile([128, ZW], f16)
            mylab = cp.tile([128, T], f32)

            # Chunk-index-major consumption (below) means each landed slice
            # unlocks ~17us of DVE work across all 8 tiles, so the two DMA
            # queues stay ahead after chunk 0. Order: fast-start pair first,
            # then x on sync / sqb on scalar in chunk order, chunk-0 split
            # so tile0's remainder starts early.
            nc.sync.dma_start(m2a[:], m2slab_d.ap()[:, 0:384])
            # chunk-0 slices in four pieces per queue (subtile deps are
            # tracked, so tile0's first ops start as soon as their piece
            # lands - no duplicate fast-start tiles needed)
            for lo, hi in ((0, FW), (FW, 1024), (1024, 2176)):
                nc.sync.dma_start(xts[0][:, lo:hi], xTrot_d.ap()[:, lo:hi])
                nc.scalar.dma_start(
                    sqbs[0][:, lo:hi],
                    sqrot_d.ap()[:, lo:hi].broadcast_to([128, hi - lo]),
                )
            nc.sync.dma_start(xts[0][:, 2176:SW], xTrot_d.ap()[:, 2176:SW])
            nc.scalar.dma_start(
                sqbs[0][:, 2176:SW],
                sqrot_d.ap()[:, 2176:SW].broadcast_to([128, SW - 2176]),
            )
            nc.scalar.dma_start(m2rest[:], m2slab_d.ap()[:, 384:R])
            for c in range(1, 4):
                nc.sync.dma_start(
                    xts[c][:], xTrot_d.ap()[:, c * CW : c * CW + SW]
                )
                nc.scalar.dma_start(
                    sqbs[c][:],
                    sqrot_d.ap()[:, c * CW : c * CW + SW].broadcast_to([128, SW]),
                )
            nc.sync.dma_start(labz[:], labz_d.ap().broadcast_to([128, ZW]))
            nc.sync.dma_start(mylab[:], mylab_d.ap())
            nc.scalar.dma_start(sqrow[:], sqrot_d.ap())

            ones_row = cp.tile([1, 128], f16)
            nc.vector.memset(ones_row[:], 1.0)
            zeros256 = cp.tile([128, WINW], f16)
            nc.vector.memset(zeros256[:], 0.0)
            zeros1024 = cp.tile([128, CW // 2], f16)
            nc.vector.memset(zeros1024[:], 0.0)

            with (
                tc.tile_pool(name="psum", bufs=2, space="PSUM") as pp,
                tc.tile_pool(name="dum", bufs=2) as dp,
                tc.tile_pool(name="st", bufs=3) as stp,
                tc.tile_pool(name="acc", bufs=28) as acp,
            ):
                chains = {t: [None, None] for t in range(T)}
                nch = {t: 0 for t in range(T)}  # accum-op parity per tile

                def addmin(t, ps_ap, sq_ap, wd, tag):
                    dum = dp.tile([128, wd], f16, tag=tag)
                    nacc = acp.tile([128, 1], f32, tag="acc")
                    k = nch[t] % 2
                    nch[t] += 1
                    nc.vector._custom_dve(
                        op_addmin,
                        out=dum[:],
                        in0=ps_ap,
                        in1=sq_ap,
                        s0=0.0,
                        s1=(1e30 if chains[t][k] is None else chains[t][k][:]),
                        imm2=0.0,
                        accum_out=nacc[:],
                    )
                    chains[t][k] = nacc

                for c in range(4):
                    for t in range(T):
                        base = 128 * t
                        w = (
                            m2a[:, base : base + 128]
                            if t < 3
                            else m2rest[:, base - 384 : base - 256]
                        )
                        if c == 0 and t == 0:
                            # fast-start: narrow first chunk, then the
                            # remainder in two pieces matching the DMA splits
                            ps = pp.tile([128, FW], f32, tag="ps")
                            nc.tensor.matmul(
                                ps[:], w, xts[0][:, 0:FW], start=True, stop=True
                            )
                            addmin(0, ps[:], sqbs[0][:, 0:FW], FW, "dumf")
                            for lo, hi in ((FW, 1024), (1024, CW)):
                                ps = pp.tile([128, hi - lo], f32, tag="ps")
                                for q in range((hi - lo) // 512):
                                    o = lo + 512 * q
                                    nc.tensor.matmul(
                                        ps[:, 512 * q : 512 * q + 512],
                                        w,
                                        xts[0][:, o : o + 512],
                                        start=True,
                                        stop=True,
                                    )
                                addmin(
                                    0, ps[:], sqbs[0][:, lo:hi], hi - lo, "dum"
                                )
                            continue
                        ps = pp.tile([128, CW], f32, tag="ps")
                        nq = 4 if c < 3 else 3
                        for q in range(nq):
                            o = base + 512 * q
                            nc.tensor.matmul(
                                ps[:, 512 * q : 512 * q + 512],
                                w,
                                xts[c][:, o : o + 512],
                                start=True,
                                stop=True,
                            )
                        if c < 3:
                            addmin(
                                t, ps[:], sqbs[c][:, base : base + CW], CW, "dum"
                            )
                            continue
                        # chunk 3: plain 1536 + eq tail 512 (K=1 sq matmul)
                        nc.tensor.matmul(
                            ps[:, CW - EQW : CW],
                            ones_row[:],
                            sqrow[0:1, EO + base : EO + base + EQW],
                            start=True,
                            stop=False,
                        )
                        nc.tensor.matmul(
                            ps[:, CW - EQW : CW],
                            w,
                            xts[3][:, base + CW - EQW : base + CW],
                            start=False,
                            stop=True,
                        )
                        pw = CW - EQW  # 1536
                        # stage the eq tail to SBUF fp16 on the (idle) scalar
                        # engine so the big psum buf frees right after the
                        # plain reduction; otherwise the CAND ops hold it and
                        # the PE pipeline bubbles ~1.5us/tile (measured).
                        eqst = dp.tile([128, EQW], f16, tag="eqst")
                        nc.scalar.activation(eqst[:], ps[:, pw:CW], Act.Copy)
                        addmin(t, ps[:, 0:pw], sqbs[3][:, base : base + pw], pw, "dum")
                        # prewindow half of the eq zone: all different-label
                        addmin(t, eqst[:, 0:WINW], zeros256[:], WINW, "dz")
                        mg = acp.tile([128, 1], f32, tag="mg")
                        nc.vector.tensor_tensor(
                            mg[:], chains[t][0][:], chains[t][1][:], op=Alu.min
                        )
                        # eq-masked min over the true window -> hardest negative
                        ed = dp.tile([128, WINW], f16, tag="eqd")
                        nm = acp.tile([128, 1], f32, tag="nm")
                        nc.vector._custom_dve(
                            op_min,
                            out=ed[:],
                            in0=eqst[:, WINW:EQW],
                            in1=labz[:, base + WINW : base + EQW],
                            s0=mylab[:, t : t + 1],
                            s1=mg[:],
                            imm2=BIGB,
                            accum_out=nm[:],
                        )
                        nc.scalar.dma_start(out_d.ap()[:, t : t + 1], nm[:])
                        # eq-masked max over the true window -> hardest positive
                        ed2 = dp.tile([128, WINW], f16, tag="eqd2")
                        pm = acp.tile([128, 1], f32, tag="pm")
                        nc.vector._custom_dve(
                            op_max,
                            out=ed2[:],
                            in0=eqst[:, WINW:EQW],
                            in1=labz[:, base + WINW : base + EQW],
                            s0=mylab[:, t : t + 1],
                            s1=-1e30,
                            imm2=BIGB,
                            accum_out=pm[:],
                        )
                        nc.scalar.dma_start(out_d.ap()[:, T + t : T + t + 1], pm[:])

    nc.compile()
    return nc


def _prep_inputs(x, labels, M):
    """Sort rows by label; build per-core pre-rotated, padded inputs.
    Validates that every row's label group falls inside the per-tile
    window [128*floor(r/128) - 64, 128*floor(r/128) + 192)."""
    N, D = x.shape
    R = N // M
    T = R // 128
    labels = np.asarray(labels)
    perm = np.argsort(labels, kind="stable")
    xs = np.ascontiguousarray(x[perm])
    ls = labels[perm]
    sq = (xs.astype(np.float64) ** 2).sum(1)

    # group bounds per row
    bounds = np.flatnonzero(np.diff(ls)) + 1
    starts = np.concatenate([[0], bounds])
    ends = np.concatenate([bounds, [N]])
    sizes = ends - starts
    first = np.repeat(starts, sizes)
    last = np.repeat(ends - 1, sizes)
    tf = (np.arange(N) // 128) * 128
    windows_ok = bool((first >= tf - 64).all() and (last <= tf + 191).all())

    xsT16 = np.ascontiguousarray(xs.T.astype(np.float16))  # [128, N]
    sq16 = sq.astype(np.float16)
    ls16 = ls.astype(np.float16)
    ZW = 128 * (T - 1) + EQW

    in_maps = []
    for c in range(M):
        rot0 = (c * R + 192) % N
        idx = (rot0 + np.arange(N + PAD)) % N
        zidx = (c * R - ZOFF + np.arange(ZW)) % N
        rows = c * R + np.arange(R)
        in_maps.append(
            {
                "xTrot": np.ascontiguousarray(xsT16[:, idx]),
                "m2slab": np.ascontiguousarray(
                    (-2.0 * xs[rows]).T.astype(np.float16)
                ),
                "labz": np.ascontiguousarray(ls16[zidx].reshape(1, ZW)),
                "mylab": np.ascontiguousarray(
                    ls[rows].astype(np.float32).reshape(T, 128).T
                ),
                "sqi": np.ascontiguousarray(
                    sq[rows].astype(np.float32).reshape(T, 128).T
                ),
                "sqrot": np.ascontiguousarray(sq16[idx].reshape(1, N + PAD)),
            }
        )
    return in_maps, windows_ok


def kernel(embeddings, labels):
    global LAST_RESULT
    x = np.asarray(embeddings, dtype=np.float32)
    lab = np.asarray(labels)
    N, D = x.shape
    M = 8
    assert D == 128 and N % (M * 128) == 0

    in_maps, windows_ok = _prep_inputs(x, lab, M)
    assert windows_ok, "label-group window invariant violated"
    key = (N, M)
    if key not in _NC_CACHE:
        _NC_CACHE[key] = build_nc(N, M)
    nc = _NC_CACHE[key]

    if TRACE:
        _install_ntff_hook()
    dev_maps = [{k: v for k, v in m.items() if k != "sqi"} for m in in_maps]
    res = bass_utils.run_bass_kernel_spmd(
        nc, dev_maps, core_ids=list(range(M)), trace=TRACE
    )
    LAST_RESULT = res

    R = N // M
    T = R // 128
    total = 0.0
    cnt = 0.0
    for c in range(M):
        o = res.results[c]["out"].astype(np.float64)
        negmin = o[:, 0:T]
        posmax = o[:, T : 2 * T]
        sqi = in_maps[c]["sqi"].astype(np.float64)
        hp2 = posmax - BIGB + sqi
        hn2 = negmin + sqi
        valid = (hp2 > TAU) & (hn2 < BIGB / 2.0)
        hp = np.sqrt(np.maximum(hp2, 0.0))
        hn = np.sqrt(np.maximum(hn2, 0.0))
        pr = np.maximum(hp + MARGIN - hn, 0.0) * valid
        total += pr.sum()
        cnt += valid.sum()
    loss = total / max(cnt, 1.0) if cnt > 0 else 0.0
    return np.float32(loss)


def _install_ntff_hook():
    """The container's antenv stub lacks axon_hooks; provide it so
    run_bass_kernel_spmd(trace=True) can capture NTFF profiles."""
    import contextlib
    import ctypes
    import types

    try:
        from antenv.axon_hooks import get_axon_ntff_profile_hook  # noqa: F401

        return
    except ImportError:
        pass
    import antenv

    mod = types.ModuleType("antenv.axon_hooks")
    _h = {"h": None}
    mod.set_axon_ntff_profile_hook = lambda h: _h.__setitem__("h", h)
    mod.get_axon_ntff_profile_hook = lambda: _h["h"]
    sys.modules["antenv.axon_hooks"] = mod
    antenv.axon_hooks = mod

    so_path = "/opt/axon/libaxon_pjrt.so"
    if not os.path.exists(so_path):
        return
    lib = ctypes.CDLL(so_path)
    if not hasattr(lib, "axon_start_nrt_profile"):
        return
    lib.axon_start_nrt_profile.argtypes = [
        ctypes.POINTER(ctypes.c_int64),
        ctypes.c_size_t,
    ]
    lib.axon_start_nrt_profile.restype = ctypes.c_int64
    lib.axon_stop_nrt_profile.argtypes = [ctypes.c_char_p]
    lib.axon_stop_nrt_profile.restype = ctypes.c_int64

    @contextlib.contextmanager
    def _hook(output_dir, device_ids):
        import jax

        jax.devices()
        if device_ids:
            ids = (ctypes.c_int64 * len(device_ids))(*device_ids)
            rc = lib.axon_start_nrt_profile(ids, len(device_ids))
        else:
            rc = lib.axon_start_nrt_profile(None, 0)
        if rc != 0:
            raise RuntimeError(f"axon_start_nrt_profile rc={rc}")
        try:
            yield
        finally:
            n = lib.axon_stop_nrt_profile(str(output_dir).encode())
            print(f"profile: {n} file(s) written to {output_dir}", file=sys.stderr)

    mod.set_axon_ntff_profile_hook(_hook)

